# revision 7
# baseline (speedup 1.0000x reference)
"""Multi-head causal attention on 8 Trainium2 cores (Bass/Tile), v2.

Problem: B=4, S=2048, D=2048, H=16 heads of dim 128, causal, fp32.
  q,k,v = x@Wq, x@Wk, x@Wv (split heads); scores=q@k^T (causal mask,
  /sqrt(128)); out = softmax @ v (merged) @ Wo + bo.

Sharding (8 cores): core c -> (batch b=c//2, head-half hg=c%2).
Each core computes its batch's attention for 8 of the 16 heads plus the
partial output projection for those heads' rows of Wo. Host adds the two
partials per batch plus the bias.

v2 vs v1 (986us): all matmul operands bf16 (PSUM stays fp32), x^T fully
resident in SBUF (loaded once, not once per head-group), softmax
denominator moved off TensorE (DVE accumulate + GpSimd partition
all-reduce instead of ones-vector matmuls), context for all 8 heads kept
resident so the output projection accumulates all heads on-chip and
writes ONE fp32 [S,D] partial per core (was 4).

Per-core layout notes:
  xk[k]  [128, S]        x^T rows k*128..+128 (d on partitions)
  wq/wk/wv_t [128,16,256] per-group weight slice (d on partitions)
  kt[t]  [128, S]        K^T for head t of current group
  v2     [128, 16, 256]  V (seq on partitions, 16 seq-chunks) both heads
  qt     [128, 512]      Q^T chunk
  pt     [128, 512]      exp(S^T) tile (sk on partitions, sq free)
  ctx    [128, 8, S]     ctx^T for all 8 heads (hd on partitions)
"""

import numpy as np
import ml_dtypes

import concourse.bass as bass
import concourse.mybir as mybir
import concourse.tile as tile
from concourse import bacc, bass_isa
from concourse.bass_utils import run_bass_kernel_spmd
from concourse.masks import make_upper_triangular

F32 = mybir.dt.float32
BF16 = mybir.dt.bfloat16
EXP = mybir.ActivationFunctionType.Exp
MULT = mybir.AluOpType.mult
ADD = mybir.AluOpType.add

B, S, D = 4, 2048, 2048
HD = 128          # head dim
NH = 8            # heads per core
G = 2             # heads per group
NG = NH // G      # 4 groups
SQ = 512          # sq chunk (matmul moving dim)
NSQ = S // SQ     # 4
NK = D // 128     # 16 contraction chunks
DH = D // 2       # 1024 = per-core slice of d_out for q/k/v
SCALE = 1.0 / float(np.sqrt(HD))


def _build():
    nc = bacc.Bacc("TRN2", target_bir_lowering=False, debug=False, num_devices=8)

    xt = nc.dram_tensor("xt", [NK, 128, S], BF16, kind="ExternalInput")
    # weights pre-laid-out on host: [g, p, k, n]
    wq = nc.dram_tensor("wq", [NG, 128, NK, G * HD], BF16, kind="ExternalInput")
    wk = nc.dram_tensor("wk", [NG, 128, NK, G * HD], BF16, kind="ExternalInput")
    wv = nc.dram_tensor("wv", [NG, 128, NK, G * HD], BF16, kind="ExternalInput")
    wo = nc.dram_tensor("wo", [128, NH, D], BF16, kind="ExternalInput")
    out = nc.dram_tensor("out", [S, D], BF16, kind="ExternalOutput")

    with tile.TileContext(nc) as tc:
        with (
            tc.tile_pool(name="const", bufs=1) as constp,
            tc.tile_pool(name="xk", bufs=1) as xkp,
            tc.tile_pool(name="wqkv", bufs=1) as wpool,
            tc.tile_pool(name="ktv", bufs=2) as ktvp,
            tc.tile_pool(name="qt", bufs=4) as qtp,
            tc.tile_pool(name="pt", bufs=6) as ptp,
            tc.tile_pool(name="ctx", bufs=1) as ctxp,
            tc.tile_pool(name="den", bufs=2) as denp,
            tc.tile_pool(name="wop", bufs=2) as wop,
            tc.tile_pool(name="osb", bufs=3) as osbp,
            tc.tile_pool(name="ps_proj", bufs=3, space="PSUM") as ps_proj,
            tc.tile_pool(name="ps_st", bufs=3, space="PSUM") as ps_st,
            tc.tile_pool(name="ps_ctx", bufs=2, space="PSUM") as ps_ctx,
        ):
            tri = constp.tile([128, 128], BF16, name="tri")
            make_upper_triangular(nc, tri[:], val=1.0, diag=True)

            # First group's weights + x, ordered so the first Q-pass can
            # start as soon as wq0 and the first j=0 x-chunk land: the x
            # DMAs are chunked per (k, j) so the j=0 column block arrives
            # ~4x sooner than whole-row loads would.
            w_tiles = []
            g0 = []
            for nm, w_dr in (("wq", wq), ("wk", wk), ("wv", wv)):
                t_ = wpool.tile([128, NK, G * HD], BF16, tag=nm, name=f"{nm}0")
                g0.append((t_, w_dr))
            nc.sync.dma_start(g0[0][0][:], g0[0][1].ap()[0])

            xk = [xkp.tile([128, S], BF16, tag=f"xk{k}", name=f"xk{k}") for k in range(NK)]
            for j in range(NSQ):
                for k in range(NK):
                    nc.sync.dma_start(
                        xk[k][:, j * SQ:(j + 1) * SQ],
                        xt.ap()[k, :, j * SQ:(j + 1) * SQ],
                    )
                if j == 0:
                    nc.sync.dma_start(g0[1][0][:], g0[1][1].ap()[0])
                    nc.sync.dma_start(g0[2][0][:], g0[2][1].ap()[0])

            ctx = ctxp.tile([128, NH, S], BF16, name="ctx")

            for g in range(NG):
                if g == 0:
                    wq_t, wk_t, wv_t = (t for t, _ in g0)
                else:
                    wq_t = wpool.tile([128, NK, G * HD], BF16, tag="wq", name=f"wq{g}")
                    wk_t = wpool.tile([128, NK, G * HD], BF16, tag="wk", name=f"wk{g}")
                    wv_t = wpool.tile([128, NK, G * HD], BF16, tag="wv", name=f"wv{g}")
                    for w_sb, w_dr in ((wq_t, wq), (wk_t, wk), (wv_t, wv)):
                        nc.sync.dma_start(w_sb[:], w_dr.ap()[g])

                kt = [
                    ktvp.tile([128, S], BF16, tag=f"kt{t}", name=f"kt{g}_{t}")
                    for t in range(G)
                ]
                v2 = ktvp.tile([128, NK, G * HD], BF16, tag="v2", name=f"v2{g}")

                for j in range(NSQ):
                    # ---- pass Q: QT[t] [hd=128, sq 512]
                    qt = []
                    for t in range(G):
                        pq = ps_proj.tile([128, SQ], F32, tag="proj", name=f"pq{t}")
                        for k in range(NK):
                            nc.tensor.matmul(
                                pq[:],
                                wq_t[:, k, t * HD:(t + 1) * HD],
                                xk[k][:, j * SQ:(j + 1) * SQ],
                                start=(k == 0),
                                stop=(k == NK - 1),
                            )
                        q_ = qtp.tile([128, SQ], BF16, tag="qt", name=f"qt{t}")
                        nc.scalar.copy(q_[:], pq[:])
                        qt.append(q_)

                    # ---- pass K: KT[t][:, j*SQ:+SQ]
                    for t in range(G):
                        pk = ps_proj.tile([128, SQ], F32, tag="proj", name=f"pk{t}")
                        for k in range(NK):
                            nc.tensor.matmul(
                                pk[:],
                                wk_t[:, k, t * HD:(t + 1) * HD],
                                xk[k][:, j * SQ:(j + 1) * SQ],
                                start=(k == 0),
                                stop=(k == NK - 1),
                            )
                        nc.scalar.copy(kt[t][:, j * SQ:(j + 1) * SQ], pk[:])

                    # ---- pass V: V[sq 128, 2*HD] for 4 sq-subchunks
                    for s_ in range(4):
                        pv = ps_proj.tile([128, 256], F32, tag="proj", name=f"pv{s_}")
                        for k in range(NK):
                            nc.tensor.matmul(
                                pv[:],
                                xk[k][:, j * SQ + s_ * 128:j * SQ + (s_ + 1) * 128],
                                wv_t[:, k, :],
                                start=(k == 0),
                                stop=(k == NK - 1),
                            )
                        nc.scalar.copy(v2[:, 4 * j + s_, :], pv[:])

                    # ---- attention for both heads at this j
                    n_sk = 4 * (j + 1)
                    for t in range(G):
                        den = denp.tile([128, SQ], F32, tag="den", name="den")
                        cps = ps_ctx.tile([128, SQ], F32, tag="ctx", name="cps")
                        for i in range(n_sk):
                            r = i - 4 * j  # >=0: straddles the causal diagonal
                            lo = 128 * r if r > 0 else 0
                            st = ps_st.tile([128, SQ], F32, tag="st", name="st")
                            nc.tensor.matmul(
                                st[:, lo:],
                                kt[t][:, i * 128:(i + 1) * 128],
                                qt[t][:, lo:],
                                start=True,
                                stop=True,
                            )
                            pt = ptp.tile([128, SQ], BF16, tag="pt", name="pt")
                            nc.scalar.activation(
                                pt[:, lo:], st[:, lo:], EXP, scale=SCALE
                            )
                            if r >= 0:
                                nc.vector.tensor_tensor(
                                    pt[:, lo:lo + 128],
                                    pt[:, lo:lo + 128],
                                    tri[:],
                                    MULT,
                                )
                            if i == 0:
                                # den spans [0:SQ] only when j==0 (i==0 implies
                                # lo==0 except j==0 where the whole row range
                                # starts at the diagonal block): i==0 has r<=0
                                # for j>0 and r==0, lo==0 for j==0 -> full SQ.
                                nc.vector.tensor_copy(den[:], pt[:])
                            else:
                                nc.vector.tensor_tensor(
                                    den[:, lo:], den[:, lo:], pt[:, lo:], ADD
                                )
                            nc.tensor.matmul(
                                cps[:, lo:],
                                v2[:, i, t * HD:(t + 1) * HD],
                                pt[:, lo:],
                                start=(i == 0),
                                stop=(i == n_sk - 1),
                            )
                        red = denp.tile([128, SQ], F32, tag="red", name="red")
                        nc.gpsimd.partition_all_reduce(
                            red[:], den[:], 128, bass_isa.ReduceOp.add
                        )
                        rrep = denp.tile([128, SQ], F32, tag="rrep", name="rrep")
                        nc.vector.reciprocal_approx_fast(rrep[:], red[:])
                        h = g * G + t
                        nc.vector.tensor_tensor(
                            ctx[:, h, j * SQ:(j + 1) * SQ], cps[:], rrep[:], MULT
                        )

            # ---- output projection: out = sum_h ctx_h @ Wo_h
            for m in range(NSQ):
                wo_m = wop.tile([128, NH, SQ], BF16, tag="wo", name=f"wo{m}")
                nc.sync.dma_start(wo_m[:], wo.ap()[:, :, m * SQ:(m + 1) * SQ])
                for s_ in range(S // 128):
                    ops = ps_proj.tile([128, SQ], F32, tag="proj", name="ops")
                    for h in range(NH):
                        nc.tensor.matmul(
                            ops[:],
                            ctx[:, h, s_ * 128:(s_ + 1) * 128],
                            wo_m[:, h, :],
                            start=(h == 0),
                            stop=(h == NH - 1),
                        )
                    osb = osbp.tile([128, SQ], BF16, tag="osb", name="osb")
                    nc.vector.tensor_copy(osb[:], ops[:])
                    nc.sync.dma_start(
                        out.ap()[s_ * 128:(s_ + 1) * 128, m * SQ:(m + 1) * SQ],
                        osb[:],
                    )

    nc.compile()
    return nc


_NC = None
LAST_RESULTS = None


def _get_nc():
    global _NC
    if _NC is None:
        _NC = _build()
    return _NC


BF = ml_dtypes.bfloat16


def _prep_w(W, lo):
    # [D, DH] slice -> [NG, 128, NK, 256]: w[g, p, k, n] = W[k*128+p, lo + g*256 + n]
    Wh = np.asarray(W[:, lo:lo + DH], dtype=BF)
    return np.ascontiguousarray(
        Wh.reshape(NK, 128, NG, G * HD).transpose(2, 1, 0, 3)
    )


def kernel(x, W_q, W_k, W_v, W_o, b_o):
    x = np.asarray(x, dtype=np.float32)
    b_o = np.asarray(b_o, dtype=np.float32)

    nc = _get_nc()
    in_maps = []
    for c in range(8):
        b, hg = divmod(c, 2)
        lo = hg * DH
        xtc = np.ascontiguousarray(
            np.asarray(x[b].T, dtype=BF).reshape(NK, 128, S)
        )
        woc = np.ascontiguousarray(
            np.asarray(W_o[lo:lo + DH, :], dtype=BF)
            .reshape(NH, 128, D)
            .transpose(1, 0, 2)
        )
        in_maps.append(
            {
                "xt": xtc,
                "wq": _prep_w(W_q, lo),
                "wk": _prep_w(W_k, lo),
                "wv": _prep_w(W_v, lo),
                "wo": woc,
            }
        )

    res = run_bass_kernel_spmd(nc, in_maps, core_ids=list(range(8)))
    global LAST_RESULTS
    LAST_RESULTS = res

    out = np.zeros((B, S, D), dtype=np.float32)
    for c in range(8):
        b = c // 2
        out[b] += np.asarray(res.results[c]["out"], dtype=np.float32)
    out += b_o[None, None, :]
    return out
